# revision 13
# baseline (speedup 1.0000x reference)
"""Distributed multi-head attention kernel for 8 TRN2 NeuronCores, v2.

Problem: x(4,2048,1024) -> qkv proj (w_qkv 3072x1024) -> 16-head attention
(head_dim 64, softmax) -> out proj (w_out 1024x1024 + b_out).

Sharding: head-parallel. Core c owns heads {2c, 2c+1}: it computes Q/K/V for
those heads over all 8192 tokens, runs attention, then a per-batch AllToAll
(1MB bf16) converts the head-sharded attention output into a token-sharded
layout (256 tokens/core/batch, all 16 heads) for the output projection.

v2 restructure vs v1: the ScalarE exp stream (~264us of activations) is the
pacing resource; v1 idled it ~7.5us at every QKV token-tile and ~18us at batch
boundaries because QKV(b+1)/outproj(b-1) ran as serial phases. v2 emits them
as "filler" units (QK half-tile, V half-tile, outproj m-tile) interleaved into
attention(b)'s exp-paced group stream every 3 groups, sharing the 3-bank
"big3" PSUM ring with the S tiles. q-tile finishing carries across batch
boundaries so the exp stream never drains.
"""

import numpy as np
import ml_dtypes

import concourse.bass as bass
import concourse.mybir as mybir
import concourse.tile as tile
from concourse import bacc, bass_utils
from concourse.tile import add_dep_helper

FP32 = mybir.dt.float32
BF16 = mybir.dt.bfloat16
AF = mybir.ActivationFunctionType

N_CORES = 8
B, NTOK, D = 4, 2048, 1024
T = B * NTOK  # 8192 tokens total
NH, HD = 16, 64
HL = NH // N_CORES  # 2 heads per core
SCALE = float(HD) ** -0.5  # 0.125
TN = 512  # token tile for QKV / q tile for attention
NT = T // TN  # 16
KC = D // 128  # 8 contraction chunks for projections
KT = NTOK // 128  # 16 k-chunks per batch in attention
TPB = NTOK // N_CORES  # 256 tokens per (core, batch) after A2A
TPC = T // N_CORES  # 1024 tokens per core total
WCOLS = 3 * HL * HD  # 384 qkv output dims per core


def build_nc(debug=False):
    nc = bacc.Bacc(
        "TRN2", target_bir_lowering=False, debug=False, num_devices=N_CORES
    )
    xt = nc.dram_tensor("xt", [D, T], BF16, kind="ExternalInput").ap()
    wt = nc.dram_tensor("wt", [D, WCOLS], BF16, kind="ExternalInput").ap()
    wo = nc.dram_tensor("wo", [D, D], BF16, kind="ExternalInput").ap()
    bias = nc.dram_tensor("bias", [1, D], FP32, kind="ExternalInput").ap()
    # row r of out = batch r//TPB, token (core * TPB + r % TPB) of that batch
    out = nc.dram_tensor("out", [TPC, D], FP32, kind="ExternalOutput").ap()

    with tile.TileContext(nc) as tc:
        with (
            tc.tile_pool(name="const", bufs=1) as const,
            tc.tile_pool(name="xin", bufs=24) as xin,
            tc.tile_pool(name="probs", bufs=6) as probs,
            tc.tile_pool(name="norm", bufs=6) as norm,
            tc.tile_pool(name="ot", bufs=6) as otp,
            tc.tile_pool(name="osb", bufs=3) as osbp,
            tc.tile_pool(name="fin", bufs=4) as fin,
            tc.tile_pool(name="psum", bufs=2, space="PSUM") as psum,
            tc.tile_pool(name="dram", bufs=1, space="DRAM") as dram,
        ):
            # ---- persistent SBUF state ----
            w_sb = const.tile([128, KC * WCOLS], BF16)
            nc.sync.dma_start(
                w_sb[:].rearrange("p (kc j) -> p kc j", kc=KC),
                wt.rearrange("(kc p) j -> p kc j", p=128),
            )
            wo_sb = const.tile([128, KC * D], BF16)
            b_row = const.tile([1, D], FP32)
            bias_sb = const.tile([128, D], FP32)

            def load_wo():
                # deferred until after batch 0's x-tile DMAs: wo/bias are not
                # read until the first out-proj (~100us in)
                nc.sync.dma_start(
                    wo_sb[:].rearrange("p (kc n) -> p kc n", kc=KC),
                    wo.rearrange("(kc p) n -> p kc n", p=128),
                )
                nc.sync.dma_start(b_row[:], bias[:])
                nc.gpsimd.partition_broadcast(bias_sb[:], b_row[:])

            q_sb = const.tile([128, T], BF16)  # [2 heads x 64, tokens] scaled
            k_sb = const.tile([128, T], BF16)
            # V token-major: [128 tok-in-chunk, (global chunk, head) x 65]
            v_sb = const.tile([128, (T // 128) * HL * 65], BF16)
            v3 = v_sb[:].rearrange("p (blk e) -> p blk e", e=65)
            nc.vector.memset(v3[:, :, 64:65], 1.0)

            a2a_in = {}
            a2a_out = {}
            for b in range(B - 1):
                a2a_in[b] = dram.tile(
                    [N_CORES, HL * HD, TPB], BF16, name=f"a2a_in{b}"
                )
                a2a_out[b] = dram.tile(
                    [N_CORES, HL * HD, TPB], BF16, name=f"a2a_out{b}"
                )
            # last batch: two half-size pieces so its collective and out-proj
            # overlap the tail of attention instead of serializing after it
            a2a_in3 = {}
            a2a_out3 = {}
            for hf in range(2):
                a2a_in3[hf] = dram.tile(
                    [N_CORES, HL * HD, 128], BF16, name=f"a2a_in3_{hf}"
                )
                a2a_out3[hf] = dram.tile(
                    [N_CORES, HL * HD, 128], BF16, name=f"a2a_out3_{hf}"
                )

            def emit_a2a(b):
                nc.gpsimd.collective_compute(
                    "AllToAll",
                    mybir.AluOpType.bypass,
                    replica_groups=[list(range(N_CORES))],
                    ins=[a2a_in[b].opt()],
                    outs=[a2a_out[b].opt()],
                )

            def emit_a2a3(hf):
                nc.gpsimd.collective_compute(
                    "AllToAll",
                    mybir.AluOpType.bypass,
                    replica_groups=[list(range(N_CORES))],
                    ins=[a2a_in3[hf].opt()],
                    outs=[a2a_out3[hf].opt()],
                )

            slot_list = [(kc, h) for kc in range(KT) for h in range(HL)]
            groups = [slot_list[g0 : g0 + 3] for g0 in range(0, len(slot_list), 3)]

            pending = []  # (b, group, p_t, pv) with S+exp emitted, PV not

            def emit_pv_flush():
                b, group, p_t, pv = pending.pop(0)
                for i, (kc, h) in enumerate(group):
                    gc = b * KT + kc
                    nc.tensor.matmul(
                        pv[h][0:65, :],
                        lhsT=v3[:, gc * HL + h, :],
                        rhs=p_t[:, i * 512 : (i + 1) * 512],
                        start=(kc == 0),
                        stop=(kc == KT - 1),
                    )

            def emit_group(b, group, pv, qt):
                # S matmuls + exp for this group; the PV matmuls are emitted
                # one group later (via pending) so the in-order PE queue never
                # head-stalls waiting on the exp of its own group
                q_off = b * NTOK + qt * TN
                width = len(group) * 512
                s_t = psum.tile([128, 1536], FP32, tag="big3", name="s_t")
                for i, (kc, h) in enumerate(group):
                    nc.tensor.matmul(
                        s_t[:, i * 512 : (i + 1) * 512],
                        lhsT=k_sb[
                            h * 64 : (h + 1) * 64,
                            b * NTOK + kc * 128 : b * NTOK + (kc + 1) * 128,
                        ],
                        rhs=q_sb[h * 64 : (h + 1) * 64, q_off : q_off + TN],
                        start=True,
                        stop=True,
                    )
                p_t = probs.tile([128, 1536], BF16, tag="p", name="p_t")
                nc.scalar.activation(p_t[:, 0:width], s_t[:, 0:width], AF.Exp)
                pending.append((b, group, p_t, pv))
                while len(pending) > 2:
                    emit_pv_flush()

            def finish_qt(b, pv, qt):
                for h in range(HL):
                    # single copy releases the PV PSUM bank; the rest of the
                    # normalize chain runs on SBUF off the fast path
                    o_c = norm.tile([65, 512], FP32, tag="oc", name="o_c")
                    nc.vector.tensor_copy(o_c[:], pv[h][0:65, :])
                    # reciprocal on one partition is ~3.3us (512 sequential
                    # elements); DMA-reshape the 512 denominators across 128
                    # partitions so it runs in ~4 elements/lane
                    rs = norm.tile([128, 4], FP32, tag="rs", name="rs")
                    nc.sync.dma_start(rs[:], o_c[64:65, :])
                    rr = norm.tile([128, 4], FP32, tag="rr", name="rr")
                    nc.vector.reciprocal(rr[:], rs[:])
                    rec = norm.tile([1, 512], FP32, tag="rec", name="rec")
                    nc.sync.dma_start(rec[:], rr[:])
                    bc = norm.tile([64, 512], FP32, tag="bc", name="bc")
                    nc.gpsimd.partition_broadcast(bc[:], rec[:])
                    o_t = otp.tile([64, 512], BF16, tag="o", name="o_t")
                    nc.vector.tensor_mul(o_t[:], o_c[0:64, :], bc[:])
                    if b < B - 1:
                        nc.sync.dma_start(
                            a2a_in[b][
                                2 * qt : 2 * qt + 2, h * 64 : (h + 1) * 64, :
                            ].rearrange("j p e -> p j e"),
                            o_t[:].rearrange("p (j e) -> p j e", j=2),
                        )
                    else:
                        j0 = (qt % 2) * 4
                        nc.sync.dma_start(
                            a2a_in3[qt // 2][
                                j0 : j0 + 4, h * 64 : (h + 1) * 64, :
                            ].rearrange("j p e -> p j e"),
                            o_t[:].rearrange("p (j e) -> p j e", j=4),
                        )

            # ---- filler units (emitted between attention groups) ----
            # Each unit is <= ~2.2us of PE work so it fits inside the ~3us
            # exp runway the two-group s_t pipeline provides; bigger units
            # head-block the in-order PE queue and stall the exp stream.
            xts_store = {}

            def emit_xload(t):
                # prefetch: 8 x-tile DMAs, no engine work
                xts = []
                for kc in range(KC):
                    x_t = xin.tile([128, TN], BF16, tag="xt", name="x_t")
                    nc.sync.dma_start(
                        x_t[:],
                        xt[kc * 128 : (kc + 1) * 128, t * TN : (t + 1) * TN],
                    )
                    xts.append(x_t)
                xts_store[t] = xts

            def emit_qm(t, m):
                # one 8-chunk projection chain: m=0 -> Q^T (scaled), m=1 -> K^T
                xts = xts_store[t]
                y_ps = psum.tile([128, 1536], FP32, tag="big3", name="y_qm")
                for kc in range(KC):
                    nc.tensor.matmul(
                        y_ps[:, 0:512],
                        lhsT=w_sb[
                            :,
                            kc * WCOLS + m * 128 : kc * WCOLS + (m + 1) * 128,
                        ],
                        rhs=xts[kc][:],
                        start=kc == 0,
                        stop=kc == KC - 1,
                    )
                # epilogues on VectorE (keep ScalarE free for exp)
                if m == 0:
                    nc.vector.tensor_scalar_mul(
                        q_sb[:, t * TN : (t + 1) * TN], y_ps[:, 0:512], SCALE
                    )
                else:
                    nc.vector.tensor_copy(
                        k_sb[:, t * TN : (t + 1) * TN], y_ps[:, 0:512]
                    )

            def emit_v(t):
                # V natural layout: 4 token subtiles share one PSUM bank;
                # start=True clears has_written flags bank-wide, so chain
                # ordering deps so each accumulation group finishes before
                # the next begins.
                xts = xts_store.pop(t)
                y_ps = psum.tile([128, 1536], FP32, tag="big3", name="y_v")
                prev = None
                for s in range(4):
                    for kc in range(KC):
                        st, sp = kc == 0, kc == KC - 1
                        mm = nc.tensor.matmul(
                            y_ps[:, s * 128 : (s + 1) * 128],
                            lhsT=xts[kc][:, s * 128 : (s + 1) * 128],
                            rhs=w_sb[:, kc * WCOLS + 256 : kc * WCOLS + WCOLS],
                            start=st,
                            stop=sp,
                        )
                        if prev is not None:
                            add_dep_helper(
                                mm.ins, prev.ins, sync=False,
                                reason="bank flag-clear order",
                            )
                        prev = mm
                nc.vector.tensor_copy(
                    v3[:, (t * 4) * HL : (t * 4 + 4) * HL, 0:64],
                    y_ps[:, 0:512]
                    .rearrange("p (s hd) -> p s hd", s=4)
                    .rearrange("p s (h d) -> p (s h) d", h=HL),
                )

            osb_store = {}

            def emit_op(bb, m):
                # out-proj m-tile (128 tokens) of batch bb; two 8-chunk
                # accumulation chains (nh halves) in banks 0-1 of a big3 slot
                if m == 0:
                    o_sb = osbp.tile(
                        [128, N_CORES * TPB], BF16, tag="osb", name="o_sb"
                    )
                    for i in range(N_CORES):
                        nc.sync.dma_start(
                            o_sb[:, i * TPB : (i + 1) * TPB], a2a_out[bb][i, :, :]
                        )
                    osb_store[bb] = o_sb
                o_sb = osb_store[bb]
                o_ps = psum.tile([128, 1536], FP32, tag="big3", name="o_ps")
                for i in range(N_CORES):
                    for nh in range(2):
                        nc.tensor.matmul(
                            o_ps[:, nh * 512 : (nh + 1) * 512],
                            lhsT=o_sb[
                                :, i * TPB + m * 128 : i * TPB + (m + 1) * 128
                            ],
                            rhs=wo_sb[:, i * D + nh * 512 : i * D + nh * 512 + 512],
                            start=(i == 0),
                            stop=(i == N_CORES - 1),
                        )
                out_t = fin.tile([128, D], FP32, tag="outt", name="out_t")
                for nh in range(2):
                    nc.vector.tensor_add(
                        out_t[:, nh * 512 : (nh + 1) * 512],
                        o_ps[:, nh * 512 : (nh + 1) * 512],
                        bias_sb[:, nh * 512 : (nh + 1) * 512],
                    )
                nc.sync.dma_start(
                    out[bb * TPB + m * 128 : bb * TPB + (m + 1) * 128, :],
                    out_t[:],
                )

            # ---- main stream ----
            fin_q = []  # (b, pv, qt) awaiting finish, carried across batches
            fillers = []  # global deque; leftovers spill into the next batch

            for b in range(B):
                if b == 0:
                    # batch 0 prologue: QKV upfront, interleaved with qt0
                    # attention as K/V chunks become available
                    pv0 = [
                        psum.tile([128, 512], FP32, tag="pv", name=f"pv{h}")
                        for h in range(HL)
                    ]
                    # interleave: [q0 k0 g v0 g | q1 k1 g v1 g g | ...] --
                    # groups flow as soon as their K chunks exist; each
                    # group's PV flush (2 groups later) stays behind the V
                    # tile that feeds it
                    emit_xload(0)

                    def drain_b0(avail, lim):
                        while (
                            drain_b0.g < min(lim, len(groups))
                            and all(kc < avail for kc, _ in groups[drain_b0.g])
                        ):
                            emit_group(0, groups[drain_b0.g], pv0, 0)
                            drain_b0.g += 1

                    drain_b0.g = 0
                    for i, t in enumerate(range(4)):
                        if t + 1 < 4:
                            emit_xload(t + 1)
                        emit_qm(t, 0)
                        emit_qm(t, 1)
                        avail = 4 * (i + 1)
                        drain_b0(avail, drain_b0.g + 1)
                        emit_v(t)
                        drain_b0(avail, len(groups))
                    fin_q.append((0, pv0, 0))
                    load_wo()
                    qts = [1, 2, 3]
                else:
                    qts = [0, 1, 2, 3]
                    # drain leftover units from the previous batch NOW: this
                    # batch's qt0 groups depend on its QKV fillers, and the
                    # in-order PE queue would deadlock if an S matmul queued
                    # ahead of the K/V writes it waits on
                    while fillers:
                        fillers.pop(0)()

                # QKV units first (they gate the next batch's attention);
                # out-proj units last so their o_sb DMAs never head-block the
                # Sync queue on a still-flying A2A
                if b < B - 1:
                    ts = range(4 * (b + 1), 4 * (b + 1) + 4)
                    for j, t in enumerate(ts):
                        if j == 0:
                            emit_xload(t)
                        fillers += [
                            lambda t=t: emit_qm(t, 0),
                            lambda t=t: emit_qm(t, 1),
                            lambda t=t, nx=t + 1 if j < 3 else None: (
                                emit_v(t),
                                emit_xload(nx) if nx is not None else None,
                            ),
                        ]
                if b >= 1:
                    fillers += [
                        lambda bb=b - 1: emit_op(bb, 0),
                        lambda bb=b - 1: emit_op(bb, 1),
                    ]

                gcount = 0
                for qt in qts:
                    pv = [
                        psum.tile([128, 512], FP32, tag="pv", name=f"pv{h}")
                        for h in range(HL)
                    ]
                    for gi, g in enumerate(groups):
                        emit_group(b, g, pv, qt)
                        gcount += 1
                        if gi == 1 and fin_q:
                            fb, fpv, fqt = fin_q.pop(0)
                            finish_qt(fb, fpv, fqt)
                            if fb < B - 1 and fqt == 3:
                                emit_a2a(fb)
                            if fb == B - 1 and fqt == 1:
                                emit_a2a3(0)
                        # insert one filler after the 2nd group of each run of
                        # 3: the two just-emitted groups' exps form the runway
                        # that hides the filler's PE-queue occupancy
                        if (
                            gcount % 3 == 2
                            and fillers
                            and (b < B - 1 or gcount >= 24)
                        ):
                            fillers.pop(0)()
                    fin_q.append((b, pv, qt))

            # ---- tail ----
            while pending:
                emit_pv_flush()
            fb, fpv, fqt = fin_q.pop(0)
            finish_qt(fb, fpv, fqt)  # (3, pv, 3)
            emit_a2a3(1)
            for m in range(2):
                o_sbh = osbp.tile(
                    [128, N_CORES * 128], BF16, tag="osb", name="o_sbh"
                )
                for i in range(N_CORES):
                    nc.sync.dma_start(
                        o_sbh[:, i * 128 : (i + 1) * 128], a2a_out3[m][i, :, :]
                    )
                o_ps = [
                    psum.tile([128, 512], FP32, tag="pv", name=f"o_ps{nh}")
                    for nh in range(2)
                ]
                for i in range(N_CORES):
                    for nh in range(2):
                        nc.tensor.matmul(
                            o_ps[nh][:, :],
                            lhsT=o_sbh[:, i * 128 : (i + 1) * 128],
                            rhs=wo_sb[:, i * D + nh * 512 : i * D + nh * 512 + 512],
                            start=(i == 0),
                            stop=(i == N_CORES - 1),
                        )
                out_t = fin.tile([128, D], FP32, tag="outt", name="out_t")
                for nh in range(2):
                    nc.vector.tensor_add(
                        out_t[:, nh * 512 : (nh + 1) * 512],
                        o_ps[nh][:, :],
                        bias_sb[:, nh * 512 : (nh + 1) * 512],
                    )
                nc.sync.dma_start(
                    out[(B - 1) * TPB + m * 128 : (B - 1) * TPB + (m + 1) * 128, :],
                    out_t[:],
                )

    nc.compile()
    return nc


_NC_CACHE = None


def _get_nc():
    global _NC_CACHE
    if _NC_CACHE is None:
        _NC_CACHE = build_nc()
    return _NC_CACHE


def make_in_maps(x, w_qkv, w_out, b_out):
    x = np.asarray(x, dtype=np.float32)
    w_qkv = np.asarray(w_qkv, dtype=np.float32)
    w_out = np.asarray(w_out, dtype=np.float32)
    b_out = np.asarray(b_out, dtype=np.float32)

    xt_np = np.ascontiguousarray(x.reshape(T, D).T).astype(ml_dtypes.bfloat16)
    wo_np = np.ascontiguousarray(w_out.T).astype(ml_dtypes.bfloat16)
    b_np = np.ascontiguousarray(b_out.reshape(1, D))

    in_maps = []
    for c in range(N_CORES):
        rows = []
        for sec in range(3):  # q, k, v sections of w_qkv
            for hh in range(HL):
                h = HL * c + hh
                rows.append(w_qkv[sec * D + h * HD : sec * D + (h + 1) * HD, :])
        wt_np = np.ascontiguousarray(np.concatenate(rows, 0).T).astype(
            ml_dtypes.bfloat16
        )  # (1024, 384)
        in_maps.append({"xt": xt_np, "wt": wt_np, "wo": wo_np, "bias": b_np})
    return in_maps


def kernel(x, w_qkv, w_out, b_out, _trace=False, _tmpdir=None):
    in_maps = make_in_maps(x, w_qkv, w_out, b_out)
    nc = _get_nc()
    res = bass_utils.run_bass_kernel_spmd(
        nc, in_maps, core_ids=list(range(N_CORES)), trace=_trace, tmpdir=_tmpdir
    )
    # core j out rows: batches 0-2: r = b*256+u -> token b*2048 + j*256 + u;
    # batch 3 (half-split A2A): r = 768 + hf*128 + u -> 6144 + hf*1024 + j*128 + u
    full = np.empty((T, D), np.float32)
    for j in range(N_CORES):
        o = np.asarray(res.results[j]["out"], dtype=np.float32)
        for b in range(B - 1):
            full[b * NTOK + j * TPB : b * NTOK + (j + 1) * TPB] = o[
                b * TPB : (b + 1) * TPB
            ]
        for hf in range(2):
            dst = (B - 1) * NTOK + hf * 1024 + j * 128
            srcr = (B - 1) * TPB + hf * 128
            full[dst : dst + 128] = o[srcr : srcr + 128]
    kernel.last_result = res
    return full.reshape(B, NTOK, D)
